# revision 16
# baseline (speedup 1.0000x reference)
"""Trainium2 Bass kernel for nn_DXVAE (DAG-GRU graph encoder), v2.

Strategy: pure data parallel over batch (8192 -> 8 cores x 1024).
Per core, 4 batch-tiles of 256 columns, processed as TWO INTERLEAVED
STREAMS (software pipelining): while stream A runs its elementwise
phase (sigmoid/messages) on Vector/Scalar/GpSimd, stream B's matmuls
keep the PE busy (and hold it at the 2.4 GHz p-state).

Layouts (same math as v1, which was verified against the reference):
  - GRU runs feature-major: h is [128 part (feat chunk), 4, 256 cols
    (batch)]; biases folded into matmuls (ones-row / rank-1 tricks).
  - Message passing runs batch-major: per-u gate/map products
    P1=[Wg1@h|Wm1@h], P2=[Wg2@h|Wm2@h] computed once when h_u is born,
    drained to bf16 SBUF by the Scalar engine.  Messages use per-pair
    scalar_tensor_tensor with adj entries as per-partition scalars,
    accumulated per-u (small SBUF footprint so two streams fit).
  - PSUM: two rotating 2-bank pools (gates / products+transposes).

Engine assignment (from HW trace rates): STT/gm-mul/rt/h_new on Vector,
sigmoid/tanh/PSUM-drains on Scalar, (h-n)/z*(h-n)/acc-add on GpSimd.
"""

import os
import sys
import types

sys.path.insert(0, "/opt/trn_rl_repo")

import numpy as np
import ml_dtypes

# Some images lack antenv.axon_hooks; bass_utils imports it unconditionally
# when tracing is requested.  Provide a registry shim so a BASS_TRACE env
# from the caller degrades to "no trace" instead of crashing the run.
try:
    import antenv.axon_hooks  # noqa: F401
except ImportError:
    import antenv

    _hooks_mod = types.ModuleType("antenv.axon_hooks")
    _hooks_mod._hook = None

    def _set_hook(h):
        _hooks_mod._hook = h

    def _get_hook():
        return _hooks_mod._hook

    _hooks_mod.set_axon_ntff_profile_hook = _set_hook
    _hooks_mod.get_axon_ntff_profile_hook = _get_hook
    sys.modules["antenv.axon_hooks"] = _hooks_mod
    antenv.axon_hooks = _hooks_mod

import concourse.bacc as bacc
import concourse.tile as tile
import concourse.mybir as mybir
from concourse import bass_utils

BF = ml_dtypes.bfloat16

N_CORES = 8
B = 8192
BC = B // N_CORES  # 1024 batch rows per core
NN = 7
SX = 27
SX0 = 23
H = 512
H3 = 3 * H
Z = 128
BT = 256  # batch columns per tile
NSB = BT // 128  # sub-batches per tile
NBT = BC // BT  # batch tiles per core
HC = H // 128  # feature chunks per hidden vector

KNOB_SCHED = os.environ.get("DXVAE_SCHED", "eager")
KNOB_PSGATE = int(os.environ.get("DXVAE_PSGATE", "2"))
KNOB_DZ_ENG = os.environ.get("DXVAE_DZ", "v")
KNOB_ACC_ENG = os.environ.get("DXVAE_ACC", "g")
KNOB_HN = os.environ.get("DXVAE_HN", "s")  # s|v: engine draining hn PSUM
# NOTE: GpSimd (Pool) cannot access PSUM at all (BIR verifier rejects it).
KNOB_SKEW = os.environ.get("DXVAE_SKEW", "0,2,7,9")
KNOB_PGBUFS = int(os.environ.get("DXVAE_PGBUFS", "0"))  # 0 -> auto
KNOB_TAIL = os.environ.get("DXVAE_TAIL", "dz")  # dz | zh
KNOB_PSPROD = int(os.environ.get("DXVAE_PSPROD", "2"))
# accumulator chains (by target node w) whose deferred pairs run on GpSimd.
# NOTE: Pool cannot execute TensorScalar/STT with AP (pointer) scalars, so
# this must stay empty unless that changes; kept for experimentation.
KNOB_GW = frozenset(
    int(x) for x in os.environ.get("DXVAE_GW", "").split(",") if x != ""
)

bf = mybir.dt.bfloat16
f32 = mybir.dt.float32
AF = mybir.ActivationFunctionType
OP = mybir.AluOpType


# --------------------------------------------------------------------------
# Kernel builder
# --------------------------------------------------------------------------

def build_nc():
    nc = bacc.Bacc(
        "TRN2", target_bir_lowering=False, debug=False, num_devices=N_CORES
    )
    d = {}

    def din(name, shape, dt=bf):
        d[name] = nc.dram_tensor(name, list(shape), dt, kind="ExternalInput").ap()

    din("xt", (6, 28, BC))      # [v-1, 27 feats + ones row, batch]
    din("xlt", (6, 28, BC))     # masked (self-loop) variant
    din("x0t", (24, BC))        # node-0 input, 23 feats + ones row
    # adjacency coefficients: [.., 0:49] = adj[u,w] flat, [.., 49:98] =
    # (adj * adj.T)[u,w] flat (the pred*succ product, precomputed on host)
    din("adjf", (128, BC // 128, 98), f32)
    din("wc", (128, HC, H3))    # W_chh.T as [kc part, kc idx, out feat]
    din("wl", (128, HC, H3))
    din("wr", (128, HC, H3))
    din("wcx", (28, H3))        # [W_cih.T ; combined bias row]
    din("wlx", (28, H3))
    din("wrx", (24, H3))
    din("bhn", (1, 3 * H))      # hidden-side n-gate biases (c, l, r)
    din("wgm1", (128, HC, 2 * H))  # [W_gate[:, :H].T | W_map[:, :H].T] chunks
    din("wgm2", (128, HC, 2 * H))  # [W_gate[:, H:].T | W_map[:, H:].T] chunks
    din("wmustd", (128, HC, 2 * Z))  # [W_mu.T | W_std.T] chunks
    din("bmurow", (1, 2 * Z))
    din("bg0", (128, 2 * H))    # [tile(b_gate) | zeros]
    din("bgmrow", (1, 2 * H))   # [b_gate | zeros] row for P1 bias matmul
    din("ident", (128, 128))
    din("ones", (1, BT))
    out_d = nc.dram_tensor("out", [BC, 2 * Z], f32, kind="ExternalOutput").ap()

    with tile.TileContext(nc) as tc:
        _emit(nc, tc, d, out_d)
    nc.compile()
    return nc


def _emit(nc, tc, d, out_d):
    from contextlib import ExitStack

    ctx = ExitStack()
    with ctx:
        singles = ctx.enter_context(tc.tile_pool(name="singles", bufs=1))
        prodp = ctx.enter_context(tc.tile_pool(name="prod", bufs=1))
        msgp = ctx.enter_context(tc.tile_pool(name="msg", bufs=1))
        grup = ctx.enter_context(tc.tile_pool(name="gru", bufs=1))
        hp = ctx.enter_context(tc.tile_pool(name="h", bufs=1))
        xp = ctx.enter_context(tc.tile_pool(name="x", bufs=1))
        outp = ctx.enter_context(tc.tile_pool(name="outp", bufs=2))
        ps_gate = ctx.enter_context(
            tc.tile_pool(name="ps_gate", bufs=KNOB_PSGATE, space="PSUM")
        )
        ps_prod = ctx.enter_context(
            tc.tile_pool(name="ps_prod", bufs=KNOB_PSPROD, space="PSUM")
        )

        # ---- resident constants -------------------------------------------
        sg = {}
        for name, shape, dt in (
            ("wc", [128, HC, H3], bf),
            ("wl", [128, HC, H3], bf),
            ("wr", [128, HC, H3], bf),
            ("wcx", [28, H3], bf),
            ("wlx", [28, H3], bf),
            ("wrx", [24, H3], bf),
            ("bhn", [1, 3 * H], bf),
            ("wgm1", [128, HC, 2 * H], bf),
            ("wgm2", [128, HC, 2 * H], bf),
            ("wmustd", [128, HC, 2 * Z], bf),
            ("bmurow", [1, 2 * Z], bf),
            ("bg0", [128, 2 * H], bf),
            ("bgmrow", [1, 2 * H], bf),
            ("ident", [128, 128], bf),
            ("ones", [1, BT], bf),
            ("adjf", [128, BC // 128, 98], f32),
        ):
            t = singles.tile(shape, dt, tag=name, name=name)
            nc.sync.dma_start(out=t, in_=d[name])
            sg[name] = t
        h_zero = singles.tile([128, HC, BT], bf, tag="h_zero")
        nc.vector.memset(h_zero, 0.0)
        wcx = sg["wcx"]
        wlx = sg["wlx"]
        wrx = sg["wrx"]

        def gru(s, h_prev, x_t, wx_t, wh_t, bhn_row, skip_gh):
            """One GRUCell step, feature-major.  Returns h_new [128, HC, BT]."""
            # r gate: out chunks 0..3 of H3; z: 4..7; n: 8..11
            rz_sb = grup.tile([128, 2 * HC, BT], bf, tag=f"rz{s}", bufs=1)
            for gate, lo in ((0, 0), (1, HC)):
                g_ps = ps_gate.tile([128, HC, BT], f32, tag="gate")
                for c in range(HC):
                    mo = lo + c
                    nc.tensor.matmul(
                        g_ps[:, c, :],
                        wx_t[:, mo * 128 : (mo + 1) * 128],
                        x_t,
                        start=True,
                        stop=skip_gh,
                    )
                    if not skip_gh:
                        for kc in range(HC):
                            nc.tensor.matmul(
                                g_ps[:, c, :],
                                wh_t[:, kc, mo * 128 : (mo + 1) * 128],
                                h_prev[:, kc, :],
                                start=False,
                                stop=(kc == HC - 1),
                            )
                nc.scalar.activation(
                    rz_sb[:, lo : lo + HC, :], g_ps, AF.Sigmoid
                )
            # n gate: inn (x side) and hn (h side) kept separate
            hn_ps = ps_gate.tile([128, HC, BT], f32, tag="gate")
            for c in range(HC):
                mo = 2 * HC + c
                if not skip_gh:
                    for kc in range(HC):
                        nc.tensor.matmul(
                            hn_ps[:, c, :],
                            wh_t[:, kc, mo * 128 : (mo + 1) * 128],
                            h_prev[:, kc, :],
                            start=(kc == 0),
                            stop=False,
                        )
                nc.tensor.matmul(
                    hn_ps[:, c, :],
                    bhn_row[:, c * 128 : (c + 1) * 128],
                    sg["ones"][:, :],
                    start=skip_gh,
                    stop=True,
                )
            # Drain hn to SBUF off the Vector engine: V reading f32 PSUM runs
            # in 1x DVE mode (and stalls under PE PSUM-port pressure); a
            # GpSimd/Scalar drain keeps the V multiply in the fast 2x path.
            hn_sb = grup.tile([128, HC, BT], bf, tag=f"hn{s}", bufs=2)
            if KNOB_HN == "s":
                nc.scalar.activation(hn_sb, hn_ps, AF.Copy)
            else:
                nc.vector.tensor_scalar_add(hn_sb, hn_ps, 0.0)
            rt = grup.tile([128, HC, BT], bf, tag=f"rt{s}", bufs=2)
            nc.vector.tensor_mul(rt, rz_sb[:, 0:HC, :], hn_sb)
            # inn + rt accumulated in PSUM.  One accumulation group per PSUM
            # bank (= two 256-col chunks): PSUM zero regions are 2KB, so
            # interleaved start/stop groups within a bank are not allowed.
            inn_ps = ps_gate.tile([128, HC, BT], f32, tag="gate")
            for half in range(HC // 2):
                c0 = 2 * half
                for j, c in enumerate((c0, c0 + 1)):
                    mo = 2 * HC + c
                    nc.tensor.matmul(
                        inn_ps[:, c, :],
                        wx_t[:, mo * 128 : (mo + 1) * 128],
                        x_t,
                        start=(j == 0),
                        stop=False,
                    )
                for j, c in enumerate((c0, c0 + 1)):
                    nc.tensor.matmul(
                        inn_ps[:, c, :],
                        sg["ident"],
                        rt[:, c, :],
                        start=False,
                        stop=(j == 1),
                    )
            n_sb = grup.tile([128, HC, BT], bf, tag=f"rt{s}", bufs=2)
            nc.scalar.activation(n_sb, inn_ps, AF.Tanh)
            h_new = hp.tile([128, HC, BT], bf, tag=f"h{s}", bufs=2)
            if KNOB_TAIL == "zh":
                # h = z*h_prev + (1-z)*n: the first term and (1-z) are ready
                # as soon as sigmoid(z) lands -- off the tanh critical path.
                zh = grup.tile([128, HC, BT], bf, tag=f"d{s}", bufs=1)
                nc.gpsimd.tensor_mul(zh, rz_sb[:, HC : 2 * HC, :], h_prev)
                oz = grup.tile([128, HC, BT], bf, tag=f"zd{s}", bufs=1)
                nc.scalar.activation(
                    oz, rz_sb[:, HC : 2 * HC, :], AF.Copy, bias=1.0, scale=-1.0
                )
                t2 = grup.tile([128, HC, BT], bf, tag=f"t2{s}", bufs=1)
                nc.vector.tensor_mul(t2, oz, n_sb)
                nc.vector.tensor_add(h_new, zh, t2)
            else:
                dz_eng = nc.gpsimd if KNOB_DZ_ENG == "g" else nc.vector
                dd = grup.tile([128, HC, BT], bf, tag=f"d{s}", bufs=1)
                dz_eng.tensor_sub(dd, h_prev, n_sb)
                zd = grup.tile([128, HC, BT], bf, tag=f"zd{s}", bufs=1)
                dz_eng.tensor_mul(zd, rz_sb[:, HC : 2 * HC, :], dd)
                nc.vector.tensor_add(h_new, n_sb, zd)
            return h_new

        def hin_from_acc(s, bt, v, accs):
            """Transpose the finished accumulators back to feature-major."""
            hin_fm = hp.tile([128, HC, BT], bf, tag=f"hin{s}", bufs=2)
            for sb in range(NSB):
                acc = accs.pop((v, sb))
                tp = ps_prod.tile([128, 512], bf, tag="prod")
                for c in range(HC):
                    nc.tensor.transpose(
                        tp[:, c * 128 : (c + 1) * 128],
                        acc[:, c * 128 : (c + 1) * 128],
                        sg["ident"],
                    )
                tp4 = tp[:, 0 : HC * 128].rearrange("p (c q) -> p c q", c=HC)
                nc.scalar.activation(
                    hin_fm[:, :, sb * 128 : (sb + 1) * 128], tp4, AF.Copy
                )
            return hin_fm

        # Contribution schedule: pair (u, w) is the message from node u into
        # node w's accumulator.  The (u, u-1) pair is critical (feeds the
        # next stage); the rest are deferred/spread so the per-stage Vector
        # load is flat ([3,4,5,4,3,2] pairs instead of [6,5,4,3,2,1]).
        if KNOB_SCHED == "balanced":
            CONTRIB_SCHED = {
                6: [(6, 5), (6, 4), (6, 3)],
                5: [(5, 4), (6, 2), (6, 1), (6, 0)],
                4: [(4, 3), (5, 3), (5, 2), (5, 1), (5, 0)],
                3: [(3, 2), (4, 2), (4, 1), (4, 0)],
                2: [(2, 1), (3, 1), (3, 0)],
                1: [(1, 0), (2, 0)],
                0: [],
            }
            PGBUFS = 4
        else:  # eager: all contributions at product birth
            CONTRIB_SCHED = {
                v: [(v, w) for w in range(v - 1, -1, -1)] for v in range(NN)
            }
            PGBUFS = 2
        if KNOB_PGBUFS:
            PGBUFS = KNOB_PGBUFS

        def products(s, bt, v, h_v, prods):
            """Candidate messages per (u, sb): since adj entries are binary,
            the message u->w is  p*m10 + s*m01 + (p*s)*d  with
              m10 = sigmoid(g1+b) * M1          (pred-only edge)
              m01 = sigmoid(g2+b) * M2          (succ-only edge)
              m11 = sigmoid(g1+g2+b) * (M1+M2)  (both edges)
              d   = m11 - m10 - m01
            computed once per source node u, so the per-pair work is just
            three fused scalar_tensor_tensor accumulates."""
            for sb in range(NSB):
                sbs = slice(sb * 128, (sb + 1) * 128)
                # P1 = [g1 + b_gate | M1]: bias via rank-1 matmul first.
                # A single matmul output cannot cross a PSUM bank (512 f32),
                # so each product matmul writes one 512-wide half.
                p1 = ps_prod.tile([128, 1024], f32, tag="prod")
                nc.tensor.matmul(
                    p1[:, 0:H], sg["ones"][:, 0:128], sg["bgmrow"][:, 0:H],
                    start=True, stop=False,
                )
                for kc in range(HC):
                    nc.tensor.matmul(
                        p1[:, 0:H], h_v[:, kc, sbs], sg["wgm1"][:, kc, 0:H],
                        start=False, stop=(kc == HC - 1),
                    )
                    nc.tensor.matmul(
                        p1[:, H : 2 * H], h_v[:, kc, sbs],
                        sg["wgm1"][:, kc, H : 2 * H],
                        start=(kc == 0), stop=(kc == HC - 1),
                    )
                # P2 = [g2 | M2] (no bias; added where needed below)
                p2 = ps_prod.tile([128, 1024], f32, tag="prod")
                for kc in range(HC):
                    nc.tensor.matmul(
                        p2[:, 0:H], h_v[:, kc, sbs], sg["wgm2"][:, kc, 0:H],
                        start=(kc == 0), stop=(kc == HC - 1),
                    )
                    nc.tensor.matmul(
                        p2[:, H : 2 * H], h_v[:, kc, sbs],
                        sg["wgm2"][:, kc, H : 2 * H],
                        start=(kc == 0), stop=(kc == HC - 1),
                    )
                # DVE ops may read at most ONE non-scalar PSUM input, so P2
                # gets a single full-width Scalar drain; P1 stays in PSUM and
                # is consumed with one-PSUM-operand Vector ops.
                p2sb = msgp.tile([128, 2 * H], bf, tag=f"p2sb{s}", bufs=2)
                nc.scalar.activation(p2sb, p2, AF.Copy)
                g1 = msgp.tile([128, H], bf, tag=f"cg1{s}", bufs=2)
                nc.scalar.activation(g1, p1[:, 0:H], AF.Sigmoid)
                m10 = msgp.tile([128, H], bf, tag=f"m10{s}", bufs=PGBUFS)
                nc.vector.tensor_mul(m10, g1, p1[:, H : 2 * H])
                t2 = msgp.tile([128, H], bf, tag=f"ct2{s}", bufs=2)
                nc.vector.tensor_add(t2, p2sb[:, 0:H], sg["bg0"][:, 0:H])
                g2 = msgp.tile([128, H], bf, tag=f"cg2{s}", bufs=2)
                nc.scalar.activation(g2, t2, AF.Sigmoid)
                m01 = msgp.tile([128, H], bf, tag=f"m01{s}", bufs=PGBUFS)
                nc.vector.tensor_mul(m01, g2, p2sb[:, H : 2 * H])
                # m11 path: g1+b already has the bias, add raw g2
                t3 = msgp.tile([128, H], bf, tag=f"ct3{s}", bufs=2)
                nc.vector.tensor_add(t3, p1[:, 0:H], p2sb[:, 0:H])
                g3 = msgp.tile([128, H], bf, tag=f"cg3{s}", bufs=2)
                nc.scalar.activation(g3, t3, AF.Sigmoid)
                m12 = msgp.tile([128, H], bf, tag=f"cm12{s}", bufs=2)
                nc.vector.tensor_add(m12, p1[:, H : 2 * H], p2sb[:, H : 2 * H])
                m11 = msgp.tile([128, H], bf, tag=f"cm11{s}", bufs=2)
                nc.gpsimd.tensor_mul(m11, g3, m12)
                dd = msgp.tile([128, H], bf, tag=f"dd{s}", bufs=PGBUFS)
                nc.gpsimd.tensor_sub(dd, m11, m10)
                nc.gpsimd.tensor_sub(dd, dd, m01)
                prods[(v, sb)] = (m10, m01, dd)

        def contribs(s, bt, v, prods, accs):
            """acc_w += p*m10_u + s*m01_u + (p*s)*d_u per scheduled pair."""
            sched = CONTRIB_SCHED[v]
            if not sched:
                return
            for sb in range(NSB):
                sbg = bt * NSB + sb
                for u, w in sched:
                    m10, m01, dd = prods[(u, sb)]
                    pred = sg["adjf"][:, sbg, u * 7 + w : u * 7 + w + 1]
                    succ = sg["adjf"][:, sbg, w * 7 + u : w * 7 + u + 1]
                    both = sg["adjf"][:, sbg, 49 + u * 7 + w : 50 + u * 7 + w]
                    # deferred (non-critical) pairs of G-assigned accumulator
                    # chains run on GpSimd; the final (critical) pair of each
                    # chain stays on Vector.
                    eng = (
                        nc.gpsimd
                        if (w in KNOB_GW and u > w + 1)
                        else nc.vector
                    )
                    acc = accs.get((w, sb))
                    if acc is None:
                        acc = msgp.tile(
                            [128, H], bf, tag=f"acc{s}_{w}_{sb}", bufs=1
                        )
                        accs[(w, sb)] = acc
                        eng.tensor_scalar_mul(acc, m10, pred)
                    else:
                        eng.scalar_tensor_tensor(
                            acc, m10, pred, acc, OP.mult, OP.add
                        )
                    eng.scalar_tensor_tensor(
                        acc, m01, succ, acc, OP.mult, OP.add
                    )
                    eng.scalar_tensor_tensor(
                        acc, dd, both, acc, OP.mult, OP.add
                    )

        def heads(s, bt, h_0):
            """mu / softplus(std) output heads, batch-major (fused matmul)."""
            for sb in range(NSB):
                sbg = bt * NSB + sb
                o_ps = ps_prod.tile([128, 1024], f32, tag="prod")
                nc.tensor.matmul(
                    o_ps[:, 0 : 2 * Z],
                    sg["ones"][:, 0:128],
                    sg["bmurow"],
                    start=True,
                    stop=False,
                )
                for kc in range(HC):
                    nc.tensor.matmul(
                        o_ps[:, 0 : 2 * Z],
                        h_0[:, kc, sb * 128 : (sb + 1) * 128],
                        sg["wmustd"][:, kc, :],
                        start=False,
                        stop=(kc == HC - 1),
                    )
                out_sb = outp.tile([128, 2 * Z], f32, tag=f"out{s}")
                nc.scalar.activation(out_sb[:, 0:Z], o_ps[:, 0:Z], AF.Copy)
                # softplus(x) = ln(1 + exp(x)) via Exp + add + Ln
                sp = outp.tile([128, Z], f32, tag=f"sp{s}")
                nc.scalar.activation(sp, o_ps[:, Z : 2 * Z], AF.Exp)
                nc.vector.tensor_scalar_add(sp, sp, 1.0)
                nc.scalar.activation(out_sb[:, Z : 2 * Z], sp, AF.Ln)
                nc.sync.dma_start(
                    out=out_d[sbg * 128 : (sbg + 1) * 128, :], in_=out_sb
                )

        def stage(s, bt, v, accs, prods):
            if v < NN - 1:
                h_prev = hin_from_acc(s, bt, v, accs)
            else:
                h_prev = h_zero
            if v >= 1:
                xt_t = xp.tile([28, BT], bf, tag=f"x{s}", bufs=2)
                nc.sync.dma_start(
                    out=xt_t, in_=d["xt"][v - 1, :, bt * BT : (bt + 1) * BT]
                )
                h_c = gru(
                    s, h_prev, xt_t, wcx, sg["wc"],
                    sg["bhn"][:, 0:H], skip_gh=(v == NN - 1),
                )
                xl_t = xp.tile([28, BT], bf, tag=f"x{s}", bufs=2)
                nc.sync.dma_start(
                    out=xl_t, in_=d["xlt"][v - 1, :, bt * BT : (bt + 1) * BT]
                )
                h_v = gru(
                    s, h_c, xl_t, wlx, sg["wl"],
                    sg["bhn"][:, H : 2 * H], skip_gh=False,
                )
                products(s, bt, v, h_v, prods)
                contribs(s, bt, v, prods, accs)
            else:
                x0_t = xp.tile([24, BT], bf, tag=f"x{s}", bufs=2)
                nc.sync.dma_start(
                    out=x0_t, in_=d["x0t"][:, bt * BT : (bt + 1) * BT]
                )
                h_0 = gru(
                    s, h_prev, x0_t, wrx, sg["wr"],
                    sg["bhn"][:, 2 * H : 3 * H], skip_gh=False,
                )
                heads(s, bt, h_0)

        # ---- main loop: skewed 2-wide pipeline over batch tiles -----------
        # Tile k starts at slot starts[k]; same-parity tiles never overlap,
        # so stream-slot k%2 tags are safe.  The skew pairs each tile's
        # vector-heavy first stage (6 message pairs) with its partner's
        # matmul-heavy middle stages.
        nbt = int(os.environ.get("DXVAE_NBT", NBT))
        starts = [int(x) for x in KNOB_SKEW.split(",")][:nbt]
        accs_by_tile = [dict() for _ in range(nbt)]
        prods_by_tile = [dict() for _ in range(nbt)]
        for t in range(max(st + NN for st in starts)):
            for k in range(nbt):
                j = t - starts[k]
                if 0 <= j < NN:
                    stage(k % 2, k, NN - 1 - j, accs_by_tile[k], prods_by_tile[k])


# --------------------------------------------------------------------------
# Host-side preparation
# --------------------------------------------------------------------------

def host_prep(inputs):
    """Build the replicated (weight) arrays and the full-batch activations."""
    g = {k: np.asarray(v) for k, v in inputs.items()}
    X = g["X"].astype(np.float32)
    adj = g["adj"].astype(np.float32)

    def t_chunks(w):  # [O, I] -> w.T chunked [128, I//128, O]
        wt = np.ascontiguousarray(w.T.astype(np.float32))  # [I, O]
        I, O = wt.shape
        return np.ascontiguousarray(
            wt.reshape(I // 128, 128, O).transpose(1, 0, 2)
        ).astype(BF)

    rep = {}
    rep["wc"] = t_chunks(g["W_chh"])
    rep["wl"] = t_chunks(g["W_lhh"])
    rep["wr"] = t_chunks(g["W_rhh"])

    def aug_x(wih, bih, bhh, rows):
        # [W_ih.T ; bias row], bias = b_ih + b_hh for r,z; b_ih for n
        bias = np.concatenate([(bih + bhh)[: 2 * H], bih[2 * H :]])
        return np.concatenate(
            [wih.T.astype(np.float32), bias[None, :]], axis=0
        )

    rep["wcx"] = aug_x(g["W_cih"], g["b_cih"], g["b_chh"], 28).astype(BF)
    rep["wlx"] = aug_x(g["W_lih"], g["b_lih"], g["b_lhh"], 28).astype(BF)
    rep["wrx"] = aug_x(g["W_rih"], g["b_rih"], g["b_rhh"], 24).astype(BF)
    rep["bhn"] = np.concatenate(
        [g["b_chh"][2 * H :], g["b_lhh"][2 * H :], g["b_rhh"][2 * H :]]
    )[None, :].astype(BF)
    rep["wgm1"] = np.concatenate(
        [t_chunks(g["W_gate"][:, :H]), t_chunks(g["W_map"][:, :H])], axis=2
    )
    rep["wgm2"] = np.concatenate(
        [t_chunks(g["W_gate"][:, H:]), t_chunks(g["W_map"][:, H:])], axis=2
    )
    rep["wmustd"] = np.concatenate(
        [t_chunks(g["W_mu"]), t_chunks(g["W_std"])], axis=2
    )
    rep["bmurow"] = np.concatenate([g["b_mu"], g["b_std"]])[None, :].astype(BF)
    rep["bg0"] = np.concatenate(
        [np.tile(g["b_gate"][None, :], (128, 1)), np.zeros((128, H), np.float32)],
        axis=1,
    ).astype(BF)
    rep["bgmrow"] = np.concatenate(
        [g["b_gate"], np.zeros((H,), np.float32)]
    )[None, :].astype(BF)
    rep["ident"] = np.eye(128, dtype=np.float32).astype(BF)
    rep["ones"] = np.ones((1, BT), np.float32).astype(BF)

    # full-batch activations (sliced per core later)
    xt = np.empty((6, 28, B), np.float32)
    xlt = np.empty((6, 28, B), np.float32)
    for v in range(1, NN):
        xt[v - 1, :27] = X[:, v, :].T
        xt[v - 1, 27] = 1.0
        xlt[v - 1, :27] = (X[:, v, :] * adj[:, v, v][:, None]).T
        xlt[v - 1, 27] = 1.0
    x0t = np.concatenate(
        [X[:, 0, :SX0].T, np.ones((1, B), np.float32)], axis=0
    )
    adj49 = adj.reshape(B, 49)
    both49 = (adj * adj.transpose(0, 2, 1)).reshape(B, 49)
    full = {
        "xt": xt.astype(BF),
        "xlt": xlt.astype(BF),
        "x0t": x0t.astype(BF),
        "adjf": np.concatenate([adj49, both49], axis=1),
    }
    return rep, full


def make_in_maps(rep, full):
    in_maps = []
    for core in range(N_CORES):
        lo, hi = core * BC, (core + 1) * BC
        m = dict(rep)
        m["xt"] = np.ascontiguousarray(full["xt"][:, :, lo:hi])
        m["xlt"] = np.ascontiguousarray(full["xlt"][:, :, lo:hi])
        m["x0t"] = np.ascontiguousarray(full["x0t"][:, lo:hi])
        m["adjf"] = np.ascontiguousarray(
            full["adjf"][lo:hi].reshape(BC // 128, 128, 98).transpose(1, 0, 2)
        )
        in_maps.append(m)
    return in_maps


_NC_CACHE = {}
LAST_RESULT = None


def kernel(**inputs):
    global LAST_RESULT
    if "nc" not in _NC_CACHE:
        _NC_CACHE["nc"] = build_nc()
    nc = _NC_CACHE["nc"]
    rep, full = host_prep(inputs)
    in_maps = make_in_maps(rep, full)
    trace = bool(os.environ.get("DXVAE_TRACE"))
    tmpdir = os.environ.get("DXVAE_TRACE_DIR") or None
    res = bass_utils.run_bass_kernel_spmd(
        nc, in_maps, core_ids=list(range(N_CORES)), trace=trace, tmpdir=tmpdir
    )
    LAST_RESULT = res
    out = np.concatenate([om["out"] for om in res.results], axis=0)
    return out.astype(np.float32)



# revision 23
# speedup vs baseline: 1.0599x; 1.0599x over previous
"""Trainium2 Bass kernel for nn_DXVAE (DAG-GRU graph encoder), v2.

Strategy: pure data parallel over batch (8192 -> 8 cores x 1024).
Per core, 4 batch-tiles of 256 columns, processed as TWO INTERLEAVED
STREAMS (software pipelining): while stream A runs its elementwise
phase (sigmoid/messages) on Vector/Scalar/GpSimd, stream B's matmuls
keep the PE busy (and hold it at the 2.4 GHz p-state).

Layouts (same math as v1, which was verified against the reference):
  - GRU runs feature-major: h is [128 part (feat chunk), 4, 256 cols
    (batch)]; biases folded into matmuls (ones-row / rank-1 tricks).
  - Message passing runs batch-major: per-u gate/map products
    P1=[Wg1@h|Wm1@h], P2=[Wg2@h|Wm2@h] computed once when h_u is born,
    drained to bf16 SBUF by the Scalar engine.  Messages use per-pair
    scalar_tensor_tensor with adj entries as per-partition scalars,
    accumulated per-u (small SBUF footprint so two streams fit).
  - PSUM: two rotating 2-bank pools (gates / products+transposes).

Engine assignment (from HW trace rates): STT/gm-mul/rt/h_new on Vector,
sigmoid/tanh/PSUM-drains on Scalar, (h-n)/z*(h-n)/acc-add on GpSimd.
"""

import os
import sys
import types

sys.path.insert(0, "/opt/trn_rl_repo")

import numpy as np
import ml_dtypes

# Some images lack antenv.axon_hooks; bass_utils imports it unconditionally
# when tracing is requested.  Provide a registry shim so a BASS_TRACE env
# from the caller degrades to "no trace" instead of crashing the run.
try:
    import antenv.axon_hooks  # noqa: F401
except ImportError:
    import antenv

    _hooks_mod = types.ModuleType("antenv.axon_hooks")
    _hooks_mod._hook = None

    def _set_hook(h):
        _hooks_mod._hook = h

    def _get_hook():
        return _hooks_mod._hook

    _hooks_mod.set_axon_ntff_profile_hook = _set_hook
    _hooks_mod.get_axon_ntff_profile_hook = _get_hook
    sys.modules["antenv.axon_hooks"] = _hooks_mod
    antenv.axon_hooks = _hooks_mod

import concourse.bacc as bacc
import concourse.tile as tile
import concourse.mybir as mybir
from concourse import bass_utils

BF = ml_dtypes.bfloat16

N_CORES = 8
B = 8192
BC = B // N_CORES  # 1024 batch rows per core
NN = 7
SX = 27
SX0 = 23
H = 512
H3 = 3 * H
Z = 128
BT = 256  # batch columns per tile
NSB = BT // 128  # sub-batches per tile
NBT = BC // BT  # batch tiles per core
HC = H // 128  # feature chunks per hidden vector

KNOB_SCHED = os.environ.get("DXVAE_SCHED", "eager")
KNOB_PSGATE = int(os.environ.get("DXVAE_PSGATE", "2"))
KNOB_DZ_ENG = os.environ.get("DXVAE_DZ", "v")
KNOB_ACC_ENG = os.environ.get("DXVAE_ACC", "g")
KNOB_HN = os.environ.get("DXVAE_HN", "s")  # s|v: engine draining hn PSUM
# NOTE: GpSimd (Pool) cannot access PSUM at all (BIR verifier rejects it).
KNOB_SKEW = os.environ.get("DXVAE_SKEW", "0,2,7,9")
KNOB_PGBUFS = int(os.environ.get("DXVAE_PGBUFS", "0"))  # 0 -> auto
KNOB_TAIL = os.environ.get("DXVAE_TAIL", "dz")  # dz | zh
KNOB_PSPROD = int(os.environ.get("DXVAE_PSPROD", "2"))
# accumulator chains (by target node w) whose deferred pairs run on GpSimd.
# NOTE: Pool cannot execute TensorScalar/STT with AP (pointer) scalars, so
# this must stay empty unless that changes; kept for experimentation.
KNOB_GW = frozenset(
    int(x) for x in os.environ.get("DXVAE_GW", "").split(",") if x != ""
)

bf = mybir.dt.bfloat16
f32 = mybir.dt.float32
AF = mybir.ActivationFunctionType
OP = mybir.AluOpType


# --------------------------------------------------------------------------
# Kernel builder
# --------------------------------------------------------------------------

def build_nc():
    nc = bacc.Bacc(
        "TRN2", target_bir_lowering=False, debug=False, num_devices=N_CORES
    )
    d = {}

    def din(name, shape, dt=bf):
        d[name] = nc.dram_tensor(name, list(shape), dt, kind="ExternalInput").ap()

    din("xt", (6, 28, BC))      # [v-1, 27 feats + ones row, batch]
    din("xlt", (6, 28, BC))     # masked (self-loop) variant
    din("x0t", (24, BC))        # node-0 input, 23 feats + ones row
    # adjacency coefficients, all indexed at u*7+w for the pair (u -> w):
    #   [.., 0:49]    a = pred*(1-succ)   (weight of m10)
    #   [.., 49:98]   b = succ*(1-pred)   (weight of m01)
    #   [.., 98:147]  c = pred*succ       (weight of m11)
    din("adjf", (128, BC // 128, 147), f32)
    din("wc", (128, HC, H3))    # W_chh.T as [kc part, kc idx, out feat]
    din("wl", (128, HC, H3))
    din("wr", (128, HC, H3))
    din("wcx", (28, H3))        # [W_cih.T ; combined bias row]
    din("wlx", (28, H3))
    din("wrx", (24, H3))
    din("bhn", (1, 3 * H))      # hidden-side n-gate biases (c, l, r)
    din("wgm1", (128, HC, 2 * H))  # [W_gate[:, :H].T | W_map[:, :H].T] chunks
    din("wgm2", (128, HC, 2 * H))  # [W_gate[:, H:].T | W_map[:, H:].T] chunks
    din("wmustd", (128, HC, 2 * Z))  # [W_mu.T | W_std.T] chunks
    din("bmurow", (1, 2 * Z))
    din("bg0", (128, 2 * H))    # [tile(b_gate) | zeros]
    din("bgmrow", (1, 2 * H))   # [b_gate | zeros] row for P1 bias matmul
    din("ident", (128, 128))
    din("ones", (1, BT))
    out_d = nc.dram_tensor("out", [BC, 2 * Z], f32, kind="ExternalOutput").ap()

    with tile.TileContext(nc) as tc:
        _emit(nc, tc, d, out_d)
    nc.compile()
    return nc


def _emit(nc, tc, d, out_d):
    from contextlib import ExitStack

    ctx = ExitStack()
    with ctx:
        singles = ctx.enter_context(tc.tile_pool(name="singles", bufs=1))
        prodp = ctx.enter_context(tc.tile_pool(name="prod", bufs=1))
        msgp = ctx.enter_context(tc.tile_pool(name="msg", bufs=1))
        grup = ctx.enter_context(tc.tile_pool(name="gru", bufs=1))
        hp = ctx.enter_context(tc.tile_pool(name="h", bufs=1))
        xp = ctx.enter_context(tc.tile_pool(name="x", bufs=1))
        outp = ctx.enter_context(tc.tile_pool(name="outp", bufs=2))
        ps_gate = ctx.enter_context(
            tc.tile_pool(name="ps_gate", bufs=KNOB_PSGATE, space="PSUM")
        )
        ps_prod = ctx.enter_context(
            tc.tile_pool(name="ps_prod", bufs=KNOB_PSPROD, space="PSUM")
        )

        # ---- resident constants -------------------------------------------
        sg = {}
        for name, shape, dt in (
            ("wc", [128, HC, H3], bf),
            ("wl", [128, HC, H3], bf),
            ("wr", [128, HC, H3], bf),
            ("wcx", [28, H3], bf),
            ("wlx", [28, H3], bf),
            ("wrx", [24, H3], bf),
            ("bhn", [1, 3 * H], bf),
            ("wgm1", [128, HC, 2 * H], bf),
            ("wgm2", [128, HC, 2 * H], bf),
            ("wmustd", [128, HC, 2 * Z], bf),
            ("bmurow", [1, 2 * Z], bf),
            ("bg0", [128, 2 * H], bf),
            ("bgmrow", [1, 2 * H], bf),
            ("ident", [128, 128], bf),
            ("ones", [1, BT], bf),
            ("adjf", [128, BC // 128, 147], f32),
        ):
            t = singles.tile(shape, dt, tag=name, name=name)
            nc.sync.dma_start(out=t, in_=d[name])
            sg[name] = t
        h_zero = singles.tile([128, HC, BT], bf, tag="h_zero")
        nc.vector.memset(h_zero, 0.0)
        wcx = sg["wcx"]
        wlx = sg["wlx"]
        wrx = sg["wrx"]

        def gru(s, h_prev, x_t, wx_t, wh_t, bhn_row, skip_gh):
            """One GRUCell step, feature-major.  Returns h_new [128, HC, BT]."""
            # r gate: out chunks 0..3 of H3; z: 4..7; n: 8..11
            rz_sb = grup.tile([128, 2 * HC, BT], bf, tag=f"rz{s}", bufs=1)
            for gate, lo in ((0, 0), (1, HC)):
                g_ps = ps_gate.tile([128, HC, BT], f32, tag="gate")
                for c in range(HC):
                    mo = lo + c
                    nc.tensor.matmul(
                        g_ps[:, c, :],
                        wx_t[:, mo * 128 : (mo + 1) * 128],
                        x_t,
                        start=True,
                        stop=skip_gh,
                    )
                    if not skip_gh:
                        for kc in range(HC):
                            nc.tensor.matmul(
                                g_ps[:, c, :],
                                wh_t[:, kc, mo * 128 : (mo + 1) * 128],
                                h_prev[:, kc, :],
                                start=False,
                                stop=(kc == HC - 1),
                            )
                nc.scalar.activation(
                    rz_sb[:, lo : lo + HC, :], g_ps, AF.Sigmoid
                )
            # n gate: inn (x side) and hn (h side) kept separate
            hn_ps = ps_gate.tile([128, HC, BT], f32, tag="gate")
            for c in range(HC):
                mo = 2 * HC + c
                if not skip_gh:
                    for kc in range(HC):
                        nc.tensor.matmul(
                            hn_ps[:, c, :],
                            wh_t[:, kc, mo * 128 : (mo + 1) * 128],
                            h_prev[:, kc, :],
                            start=(kc == 0),
                            stop=False,
                        )
                nc.tensor.matmul(
                    hn_ps[:, c, :],
                    bhn_row[:, c * 128 : (c + 1) * 128],
                    sg["ones"][:, :],
                    start=skip_gh,
                    stop=True,
                )
            # Drain hn to SBUF off the Vector engine: V reading f32 PSUM runs
            # in 1x DVE mode (and stalls under PE PSUM-port pressure); a
            # GpSimd/Scalar drain keeps the V multiply in the fast 2x path.
            hn_sb = grup.tile([128, HC, BT], bf, tag=f"hn{s}", bufs=2)
            if KNOB_HN == "s":
                nc.scalar.activation(hn_sb, hn_ps, AF.Copy)
            else:
                nc.vector.tensor_scalar_add(hn_sb, hn_ps, 0.0)
            rt = grup.tile([128, HC, BT], bf, tag=f"rt{s}", bufs=2)
            nc.vector.tensor_mul(rt, rz_sb[:, 0:HC, :], hn_sb)
            # inn + rt accumulated in PSUM.  One accumulation group per PSUM
            # bank (= two 256-col chunks): PSUM zero regions are 2KB, so
            # interleaved start/stop groups within a bank are not allowed.
            inn_ps = ps_gate.tile([128, HC, BT], f32, tag="gate")
            for half in range(HC // 2):
                c0 = 2 * half
                for j, c in enumerate((c0, c0 + 1)):
                    mo = 2 * HC + c
                    nc.tensor.matmul(
                        inn_ps[:, c, :],
                        wx_t[:, mo * 128 : (mo + 1) * 128],
                        x_t,
                        start=(j == 0),
                        stop=False,
                    )
                for j, c in enumerate((c0, c0 + 1)):
                    nc.tensor.matmul(
                        inn_ps[:, c, :],
                        sg["ident"],
                        rt[:, c, :],
                        start=False,
                        stop=(j == 1),
                    )
            n_sb = grup.tile([128, HC, BT], bf, tag=f"rt{s}", bufs=2)
            nc.scalar.activation(n_sb, inn_ps, AF.Tanh)
            h_new = hp.tile([128, HC, BT], bf, tag=f"h{s}", bufs=2)
            if KNOB_TAIL == "zh":
                # h = z*h_prev + (1-z)*n: the first term and (1-z) are ready
                # as soon as sigmoid(z) lands -- off the tanh critical path.
                zh = grup.tile([128, HC, BT], bf, tag=f"d{s}", bufs=1)
                nc.gpsimd.tensor_mul(zh, rz_sb[:, HC : 2 * HC, :], h_prev)
                oz = grup.tile([128, HC, BT], bf, tag=f"zd{s}", bufs=1)
                nc.scalar.activation(
                    oz, rz_sb[:, HC : 2 * HC, :], AF.Copy, bias=1.0, scale=-1.0
                )
                t2 = grup.tile([128, HC, BT], bf, tag=f"t2{s}", bufs=1)
                nc.vector.tensor_mul(t2, oz, n_sb)
                nc.vector.tensor_add(h_new, zh, t2)
            else:
                dz_eng = nc.gpsimd if KNOB_DZ_ENG == "g" else nc.vector
                dd = grup.tile([128, HC, BT], bf, tag=f"d{s}", bufs=1)
                dz_eng.tensor_sub(dd, h_prev, n_sb)
                zd = grup.tile([128, HC, BT], bf, tag=f"zd{s}", bufs=1)
                dz_eng.tensor_mul(zd, rz_sb[:, HC : 2 * HC, :], dd)
                nc.vector.tensor_add(h_new, n_sb, zd)
            return h_new

        def hin_from_acc(s, bt, v, accs):
            """Transpose the finished accumulators back to feature-major."""
            hin_fm = hp.tile([128, HC, BT], bf, tag=f"hin{s}", bufs=2)
            for sb in range(NSB):
                acc = accs.pop((v, sb))
                tp = ps_prod.tile([128, 512], bf, tag="prod")
                for c in range(HC):
                    nc.tensor.transpose(
                        tp[:, c * 128 : (c + 1) * 128],
                        acc[:, c * 128 : (c + 1) * 128],
                        sg["ident"],
                    )
                tp4 = tp[:, 0 : HC * 128].rearrange("p (c q) -> p c q", c=HC)
                nc.scalar.activation(
                    hin_fm[:, :, sb * 128 : (sb + 1) * 128], tp4, AF.Copy
                )
            return hin_fm

        # Contribution schedule: pair (u, w) is the message from node u into
        # node w's accumulator.  The (u, u-1) pair is critical (feeds the
        # next stage); the rest are deferred/spread so the per-stage Vector
        # load is flat ([3,4,5,4,3,2] pairs instead of [6,5,4,3,2,1]).
        if KNOB_SCHED == "balanced":
            CONTRIB_SCHED = {
                6: [(6, 5), (6, 4), (6, 3)],
                5: [(5, 4), (6, 2), (6, 1), (6, 0)],
                4: [(4, 3), (5, 3), (5, 2), (5, 1), (5, 0)],
                3: [(3, 2), (4, 2), (4, 1), (4, 0)],
                2: [(2, 1), (3, 1), (3, 0)],
                1: [(1, 0), (2, 0)],
                0: [],
            }
            PGBUFS = 4
        else:  # eager: all contributions at product birth
            CONTRIB_SCHED = {
                v: [(v, w) for w in range(v - 1, -1, -1)] for v in range(NN)
            }
            PGBUFS = 2
        if KNOB_PGBUFS:
            PGBUFS = KNOB_PGBUFS

        def products(s, bt, v, h_v, prods):
            """Candidate messages per (u, sb): since adj entries are binary,
            the message u->w is  p*m10 + s*m01 + (p*s)*d  with
              m10 = sigmoid(g1+b) * M1          (pred-only edge)
              m01 = sigmoid(g2+b) * M2          (succ-only edge)
              m11 = sigmoid(g1+g2+b) * (M1+M2)  (both edges)
              d   = m11 - m10 - m01
            computed once per source node u, so the per-pair work is just
            three fused scalar_tensor_tensor accumulates."""
            for sb in range(NSB):
                sbs = slice(sb * 128, (sb + 1) * 128)
                # P1 = [g1 + b_gate | M1]: bias via rank-1 matmul first.
                # A single matmul output cannot cross a PSUM bank (512 f32),
                # so each product matmul writes one 512-wide half.
                p1 = ps_prod.tile([128, 1024], f32, tag="prod")
                nc.tensor.matmul(
                    p1[:, 0:H], sg["ones"][:, 0:128], sg["bgmrow"][:, 0:H],
                    start=True, stop=False,
                )
                for kc in range(HC):
                    nc.tensor.matmul(
                        p1[:, 0:H], h_v[:, kc, sbs], sg["wgm1"][:, kc, 0:H],
                        start=False, stop=(kc == HC - 1),
                    )
                    nc.tensor.matmul(
                        p1[:, H : 2 * H], h_v[:, kc, sbs],
                        sg["wgm1"][:, kc, H : 2 * H],
                        start=(kc == 0), stop=(kc == HC - 1),
                    )
                # P2 = [g2 | M2] (no bias; added where needed below).
                # Gate half first so its drain can start two matmuls earlier.
                p2 = ps_prod.tile([128, 1024], f32, tag="prod")
                for kc in range(HC):
                    nc.tensor.matmul(
                        p2[:, 0:H], h_v[:, kc, sbs], sg["wgm2"][:, kc, 0:H],
                        start=(kc == 0), stop=(kc == HC - 1),
                    )
                for kc in range(HC):
                    nc.tensor.matmul(
                        p2[:, H : 2 * H], h_v[:, kc, sbs],
                        sg["wgm2"][:, kc, H : 2 * H],
                        start=(kc == 0), stop=(kc == HC - 1),
                    )
                # DVE ops may read at most ONE non-scalar PSUM input, so P2
                # gets Scalar half-drains; P1 stays in PSUM and is consumed
                # with one-PSUM-operand Vector ops.
                p2sb = msgp.tile([128, 2 * H], bf, tag=f"p2sb{s}", bufs=2)
                nc.scalar.activation(p2sb[:, 0:H], p2[:, 0:H], AF.Copy)
                nc.scalar.activation(
                    p2sb[:, H : 2 * H], p2[:, H : 2 * H], AF.Copy
                )
                g1 = msgp.tile([128, H], bf, tag=f"cg1{s}", bufs=2)
                nc.scalar.activation(g1, p1[:, 0:H], AF.Sigmoid)
                m10 = msgp.tile([128, H], bf, tag=f"m10{s}", bufs=PGBUFS)
                nc.vector.tensor_mul(m10, g1, p1[:, H : 2 * H])
                t2 = msgp.tile([128, H], bf, tag=f"ct2{s}", bufs=2)
                nc.vector.tensor_add(t2, p2sb[:, 0:H], sg["bg0"][:, 0:H])
                g2 = msgp.tile([128, H], bf, tag=f"cg2{s}", bufs=2)
                nc.scalar.activation(g2, t2, AF.Sigmoid)
                m01 = msgp.tile([128, H], bf, tag=f"m01{s}", bufs=PGBUFS)
                nc.vector.tensor_mul(m01, g2, p2sb[:, H : 2 * H])
                # m11 path: g1+b already has the bias, add raw g2
                t3 = msgp.tile([128, H], bf, tag=f"ct3{s}", bufs=2)
                nc.vector.tensor_add(t3, p1[:, 0:H], p2sb[:, 0:H])
                g3 = msgp.tile([128, H], bf, tag=f"cg3{s}", bufs=2)
                nc.scalar.activation(g3, t3, AF.Sigmoid)
                m12 = msgp.tile([128, H], bf, tag=f"cm12{s}", bufs=2)
                nc.vector.tensor_add(m12, p1[:, H : 2 * H], p2sb[:, H : 2 * H])
                m11 = msgp.tile([128, H], bf, tag=f"cm11{s}", bufs=PGBUFS)
                nc.vector.tensor_mul(m11, g3, m12)
                prods[(v, sb)] = (m10, m01, m11)

        def contribs(s, bt, v, prods, accs):
            """acc_w += a*m10_u + b*m01_u + c*m11_u per scheduled pair."""
            sched = CONTRIB_SCHED[v]
            if not sched:
                return
            for sb in range(NSB):
                sbg = bt * NSB + sb

                def emit(u, w, phase):
                    mk = prods[(u, sb)][phase]
                    k = 49 * phase + u * 7 + w
                    coef = sg["adjf"][:, sbg, k : k + 1]
                    acc = accs.get((w, sb))
                    if acc is None:
                        acc = msgp.tile(
                            [128, H], bf, tag=f"acc{s}_{w}_{sb}", bufs=1
                        )
                        accs[(w, sb)] = acc
                        nc.vector.tensor_scalar_mul(acc, mk, coef)
                    else:
                        nc.vector.scalar_tensor_tensor(
                            acc, mk, coef, acc, OP.mult, OP.add
                        )

                # Critical pair (into the next stage's node) start-to-finish
                # first; deferred pairs phase-major so consecutive V ops hit
                # different accumulators (no in-place RAW pipeline bubbles).
                for phase in range(3):
                    emit(*sched[0], phase)
                for phase in range(3):
                    for u, w in sched[1:]:
                        emit(u, w, phase)

        def heads(s, bt, h_0):
            """mu / softplus(std) output heads, batch-major (fused matmul)."""
            for sb in range(NSB):
                sbg = bt * NSB + sb
                o_ps = ps_prod.tile([128, 1024], f32, tag="prod")
                nc.tensor.matmul(
                    o_ps[:, 0 : 2 * Z],
                    sg["ones"][:, 0:128],
                    sg["bmurow"],
                    start=True,
                    stop=False,
                )
                for kc in range(HC):
                    nc.tensor.matmul(
                        o_ps[:, 0 : 2 * Z],
                        h_0[:, kc, sb * 128 : (sb + 1) * 128],
                        sg["wmustd"][:, kc, :],
                        start=False,
                        stop=(kc == HC - 1),
                    )
                out_sb = outp.tile([128, 2 * Z], f32, tag=f"out{s}")
                nc.scalar.activation(out_sb[:, 0:Z], o_ps[:, 0:Z], AF.Copy)
                # softplus(x) = ln(1 + exp(x)) via Exp + add + Ln
                sp = outp.tile([128, Z], f32, tag=f"sp{s}")
                nc.scalar.activation(sp, o_ps[:, Z : 2 * Z], AF.Exp)
                nc.vector.tensor_scalar_add(sp, sp, 1.0)
                nc.scalar.activation(out_sb[:, Z : 2 * Z], sp, AF.Ln)
                nc.sync.dma_start(
                    out=out_d[sbg * 128 : (sbg + 1) * 128, :], in_=out_sb
                )

        def stage(s, bt, v, accs, prods):
            if v < NN - 1:
                h_prev = hin_from_acc(s, bt, v, accs)
            else:
                h_prev = h_zero
            if v >= 1:
                xt_t = xp.tile([28, BT], bf, tag=f"x{s}", bufs=2)
                nc.sync.dma_start(
                    out=xt_t, in_=d["xt"][v - 1, :, bt * BT : (bt + 1) * BT]
                )
                h_c = gru(
                    s, h_prev, xt_t, wcx, sg["wc"],
                    sg["bhn"][:, 0:H], skip_gh=(v == NN - 1),
                )
                xl_t = xp.tile([28, BT], bf, tag=f"x{s}", bufs=2)
                nc.sync.dma_start(
                    out=xl_t, in_=d["xlt"][v - 1, :, bt * BT : (bt + 1) * BT]
                )
                h_v = gru(
                    s, h_c, xl_t, wlx, sg["wl"],
                    sg["bhn"][:, H : 2 * H], skip_gh=False,
                )
                products(s, bt, v, h_v, prods)
                contribs(s, bt, v, prods, accs)
            else:
                x0_t = xp.tile([24, BT], bf, tag=f"x{s}", bufs=2)
                nc.sync.dma_start(
                    out=x0_t, in_=d["x0t"][:, bt * BT : (bt + 1) * BT]
                )
                h_0 = gru(
                    s, h_prev, x0_t, wrx, sg["wr"],
                    sg["bhn"][:, 2 * H : 3 * H], skip_gh=False,
                )
                heads(s, bt, h_0)

        # ---- main loop: skewed 2-wide pipeline over batch tiles -----------
        # Tile k starts at slot starts[k]; same-parity tiles never overlap,
        # so stream-slot k%2 tags are safe.  The skew pairs each tile's
        # vector-heavy first stage (6 message pairs) with its partner's
        # matmul-heavy middle stages.
        nbt = int(os.environ.get("DXVAE_NBT", NBT))
        starts = [int(x) for x in KNOB_SKEW.split(",")][:nbt]
        accs_by_tile = [dict() for _ in range(nbt)]
        prods_by_tile = [dict() for _ in range(nbt)]
        for t in range(max(st + NN for st in starts)):
            for k in range(nbt):
                j = t - starts[k]
                if 0 <= j < NN:
                    stage(k % 2, k, NN - 1 - j, accs_by_tile[k], prods_by_tile[k])


# --------------------------------------------------------------------------
# Host-side preparation
# --------------------------------------------------------------------------

def host_prep(inputs):
    """Build the replicated (weight) arrays and the full-batch activations."""
    g = {k: np.asarray(v) for k, v in inputs.items()}
    X = g["X"].astype(np.float32)
    adj = g["adj"].astype(np.float32)

    def t_chunks(w):  # [O, I] -> w.T chunked [128, I//128, O]
        wt = np.ascontiguousarray(w.T.astype(np.float32))  # [I, O]
        I, O = wt.shape
        return np.ascontiguousarray(
            wt.reshape(I // 128, 128, O).transpose(1, 0, 2)
        ).astype(BF)

    rep = {}
    rep["wc"] = t_chunks(g["W_chh"])
    rep["wl"] = t_chunks(g["W_lhh"])
    rep["wr"] = t_chunks(g["W_rhh"])

    def aug_x(wih, bih, bhh, rows):
        # [W_ih.T ; bias row], bias = b_ih + b_hh for r,z; b_ih for n
        bias = np.concatenate([(bih + bhh)[: 2 * H], bih[2 * H :]])
        return np.concatenate(
            [wih.T.astype(np.float32), bias[None, :]], axis=0
        )

    rep["wcx"] = aug_x(g["W_cih"], g["b_cih"], g["b_chh"], 28).astype(BF)
    rep["wlx"] = aug_x(g["W_lih"], g["b_lih"], g["b_lhh"], 28).astype(BF)
    rep["wrx"] = aug_x(g["W_rih"], g["b_rih"], g["b_rhh"], 24).astype(BF)
    rep["bhn"] = np.concatenate(
        [g["b_chh"][2 * H :], g["b_lhh"][2 * H :], g["b_rhh"][2 * H :]]
    )[None, :].astype(BF)
    rep["wgm1"] = np.concatenate(
        [t_chunks(g["W_gate"][:, :H]), t_chunks(g["W_map"][:, :H])], axis=2
    )
    rep["wgm2"] = np.concatenate(
        [t_chunks(g["W_gate"][:, H:]), t_chunks(g["W_map"][:, H:])], axis=2
    )
    rep["wmustd"] = np.concatenate(
        [t_chunks(g["W_mu"]), t_chunks(g["W_std"])], axis=2
    )
    rep["bmurow"] = np.concatenate([g["b_mu"], g["b_std"]])[None, :].astype(BF)
    rep["bg0"] = np.concatenate(
        [np.tile(g["b_gate"][None, :], (128, 1)), np.zeros((128, H), np.float32)],
        axis=1,
    ).astype(BF)
    rep["bgmrow"] = np.concatenate(
        [g["b_gate"], np.zeros((H,), np.float32)]
    )[None, :].astype(BF)
    rep["ident"] = np.eye(128, dtype=np.float32).astype(BF)
    rep["ones"] = np.ones((1, BT), np.float32).astype(BF)

    # full-batch activations (sliced per core later)
    xt = np.empty((6, 28, B), np.float32)
    xlt = np.empty((6, 28, B), np.float32)
    for v in range(1, NN):
        xt[v - 1, :27] = X[:, v, :].T
        xt[v - 1, 27] = 1.0
        xlt[v - 1, :27] = (X[:, v, :] * adj[:, v, v][:, None]).T
        xlt[v - 1, 27] = 1.0
    x0t = np.concatenate(
        [X[:, 0, :SX0].T, np.ones((1, B), np.float32)], axis=0
    )
    # coefficient planes, all indexed at u*7+w for the pair (u -> w):
    # pred = adj[u, w], succ = adj[w, u], both = pred*succ
    pred49 = adj.reshape(B, 49)
    succ49 = np.ascontiguousarray(adj.transpose(0, 2, 1)).reshape(B, 49)
    both49 = pred49 * succ49
    full = {
        "xt": xt.astype(BF),
        "xlt": xlt.astype(BF),
        "x0t": x0t.astype(BF),
        "adjf": np.concatenate(
            [pred49 - both49, succ49 - both49, both49], axis=1
        ),
    }
    return rep, full


def make_in_maps(rep, full):
    in_maps = []
    for core in range(N_CORES):
        lo, hi = core * BC, (core + 1) * BC
        m = dict(rep)
        m["xt"] = np.ascontiguousarray(full["xt"][:, :, lo:hi])
        m["xlt"] = np.ascontiguousarray(full["xlt"][:, :, lo:hi])
        m["x0t"] = np.ascontiguousarray(full["x0t"][:, lo:hi])
        m["adjf"] = np.ascontiguousarray(
            full["adjf"][lo:hi].reshape(BC // 128, 128, 147).transpose(1, 0, 2)
        )
        in_maps.append(m)
    return in_maps


_NC_CACHE = {}
LAST_RESULT = None


def kernel(**inputs):
    global LAST_RESULT
    if "nc" not in _NC_CACHE:
        _NC_CACHE["nc"] = build_nc()
    nc = _NC_CACHE["nc"]
    rep, full = host_prep(inputs)
    in_maps = make_in_maps(rep, full)
    trace = bool(os.environ.get("DXVAE_TRACE"))
    tmpdir = os.environ.get("DXVAE_TRACE_DIR") or None
    res = bass_utils.run_bass_kernel_spmd(
        nc, in_maps, core_ids=list(range(N_CORES)), trace=trace, tmpdir=tmpdir
    )
    LAST_RESULT = res
    out = np.concatenate([om["out"] for om in res.results], axis=0)
    return out.astype(np.float32)

